# revision 19
# baseline (speedup 1.0000x reference)
"""CRF log-likelihood loss kernel for Trainium2 (8 NeuronCores, Bass/Tile).

Strategy (data-parallel over batch, per sharding hint):
  - B=256 batch rows sharded 32 per core; W/b/CRF tables replicated.
  - Bulk: X = exp(W^T e + b - PRE) computed as 16 big bf16 matmuls
    (1024-token chunks, PSUM-accumulated over the two 128-row h-halves)
    + ACT Exp with bias fused.  Runs dense up front so the PE p-state
    ramps to full speed.  Emissions are pre-transposed/cast to bf16 on
    host; fwd tokens ascending and bwd tokens descending are separate
    streams so chain consumption is position-aligned.
  - Chain: fwd alpha from t=0 and bwd beta from t=511 as two interleaved
    32-partition recurrences (one [32,32] matmul + one elementwise
    multiply each per round, phase-locked so PE/DVE overlap).  A constant
    prescale exp(-PRE) per step is folded into the bias so NO
    renormalization is needed (fp32/bf16 exponent range absorbs the
    drift); host adds T*PRE back to logZ exactly.
  - Host finishes: numerator (gold-path score incl. emission term) in
    numpy fp64; logZ_b = log(alpha_255^T E (x*beta)_256) + T*PRE;
    result = sum_b(score_b - logZ_b), summed over cores.
"""

import numpy as np

B, T, H, K = 256, 512, 256, 32
NCORES = 8
BS = B // NCORES          # 32 batch rows per core
NT = T * BS               # 16384 tokens per core
HALF = NT // 2            # 8192 tokens per direction
CTOK = 1024               # tokens per bulk chunk
NCH = HALF // CTOK        # 8 chunks per direction
TS_PER_CH = CTOK // BS    # 32 t-steps per chunk
NROUND = 255              # fwd t=1..255, bwd t=510..256
PRE = 5.75 * 0.6931471805599453  # per-step prescale (log-domain)

_BUILT = {}
LAST_RESULTS = None


def _build_nc():
    import concourse.bacc as bacc
    import concourse.tile as tile
    from concourse import mybir
    from contextlib import ExitStack

    f32 = mybir.dt.float32
    bf16 = mybir.dt.bfloat16
    Exp = mybir.ActivationFunctionType.Exp
    mult = mybir.AluOpType.mult

    nc = bacc.Bacc("TRN2", target_bir_lowering=False, debug=False,
                   num_devices=NCORES)

    # [128, 2 (h-half), 8192] bf16 per direction; fwd t asc, bwd t desc.
    emisF = nc.declare_dram_parameter("emisF", [128, 2, HALF], bf16,
                                      isOutput=False)
    emisB = nc.declare_dram_parameter("emisB", [128, 2, HALF], bf16,
                                      isOutput=False)
    # cbig: cols [w0 | w1 | ehat | ebwd]; cflt: cols [b-PRE, e^start, e^end]
    cbig = nc.declare_dram_parameter("cbig", [128, 4 * K], bf16,
                                     isOutput=False)
    cflt = nc.declare_dram_parameter("cflt", [K, 4], f32, isOutput=False)
    amid_d = nc.declare_dram_parameter("amid", [K, BS], bf16, isOutput=True)
    vmid_d = nc.declare_dram_parameter("vmid", [K, BS], bf16, isOutput=True)

    with ExitStack() as ctx:
        tc = ctx.enter_context(tile.TileContext(nc))
        consts = ctx.enter_context(tc.tile_pool(name="consts", bufs=1))
        emis_pool = ctx.enter_context(tc.tile_pool(name="emis", bufs=8))
        xpool = ctx.enter_context(tc.tile_pool(name="xp", bufs=2 * NCH))
        apool = ctx.enter_context(tc.tile_pool(name="ap", bufs=8))
        psum_l = ctx.enter_context(tc.tile_pool(name="pl", bufs=4,
                                                space="PSUM"))
        psum_c = ctx.enter_context(tc.tile_pool(name="pc", bufs=2,
                                                space="PSUM"))

        # constants: packed into 2 DMAs on the (otherwise idle) scalar
        # queue so the emission chunk DMAs lead the sync/gpsimd queues.
        cb_sb = consts.tile([128, 4 * K], bf16)
        cf_sb = consts.tile([K, 4], f32)
        nc.scalar.dma_start(out=cb_sb, in_=cbig[:, :])
        nc.scalar.dma_start(out=cf_sb, in_=cflt[:, :])
        w0 = cb_sb[:, 0:K]
        w1 = cb_sb[:, K:2 * K]
        ehat_sb = cb_sb[0:K, 2 * K:3 * K]
        ebwd_sb = cb_sb[0:K, 3 * K:4 * K]
        b_sb = cf_sb[:, 0:1]
        estart_sb = cf_sb[:, 1:2]
        eend_sb = cf_sb[:, 2:3]

        # ---- bulk: X = exp(logits + b - PRE), 1024-token chunks.
        # Chunk 0 is produced up front; chunks 1..7 interleave into the
        # chain (one PE matmul per job slot) so DMA/PE bulk overlaps the
        # latency-bound recurrence. ----
        xF = [None] * NCH
        xB = [None] * NCH

        def emit_dma(c, spread=False):
            # 4 subtile DMAs (F s0, F s1, B s0, B s1), each [128, 2x512]
            # (h0|h1).  spread=True fans them over 4 queues (chunk 0).
            qs = ((nc.sync, nc.scalar, nc.gpsimd, nc.scalar) if spread
                  else (nc.sync, nc.sync, nc.gpsimd, nc.gpsimd))
            sub = []
            for i, (src, nm) in enumerate(((emisF, "ef"), (emisB, "eb"))):
                xt = xpool.tile([K, CTOK], bf16, name="xt", tag="xt")
                for s in range(2):
                    off = c * CTOK + s * 512
                    e = emis_pool.tile([128, 1024], bf16, name=nm, tag=nm)
                    qs[2 * i + s].dma_start(
                        out=e, in_=src[:, :, off:off + 512])
                    sub.append(e)
                (xF if i == 0 else xB)[c] = (sub[2 * i:], xt)

        def mm_jobs(c):
            # 8 single-matmul jobs for chunk c (F/B x 2 subtiles x 2 h)
            for xs in (xF, xB):
                sub, xt = xs[c]
                for s in range(2):
                    e = sub[s]
                    ss = slice(s * 512, (s + 1) * 512)
                    pl = psum_l.tile([K, 512], f32, name="pl", tag="pl")

                    def j1(pl=pl, e=e):
                        nc.tensor.matmul(pl, w0, e[:, :512],
                                         start=True, stop=False)

                    def j2(pl=pl, e=e, xt=xt, ss=ss):
                        nc.tensor.matmul(pl, w1, e[:, 512:],
                                         start=False, stop=True)
                        nc.scalar.activation(out=xt[:, ss], in_=pl,
                                             func=Exp, bias=b_sb)

                    yield j1
                    yield j2

        emit_dma(0, spread=True)
        for job in mm_jobs(0):
            job()
        emit_dma(1)

        def bulk_jobs():
            for c in range(1, NCH):
                if c + 1 < NCH:
                    emit_dma(c + 1)
                yield from mm_jobs(c)

        jobs = bulk_jobs()

        def xsl(xs, r):
            c, o = r // TS_PER_CH, r % TS_PER_CH
            return xs[c][1][:, o * BS:(o + 1) * BS]

        # ---- chain init: a0 = X_f(0)*estart ; v0 = X_b(0)*eend ----
        a_prev = apool.tile([K, BS], bf16, name="af", tag="af")
        nc.vector.tensor_scalar(out=a_prev, in0=xsl(xF, 0),
                                scalar1=estart_sb, scalar2=None, op0=mult)
        v_prev = apool.tile([K, BS], bf16, name="av", tag="av")
        nc.vector.tensor_scalar(out=v_prev, in0=xsl(xB, 0),
                                scalar1=eend_sb, scalar2=None, op0=mult)

        # ---- 255 interleaved rounds (bulk jobs fill early PE gaps) ----
        for r in range(1, NROUND + 1):
            if r % 2 == 1:
                job = next(jobs, None)
                if job is not None:
                    job()
            pcf = psum_c.tile([K, BS], f32, name="pcf", tag="pcf")
            nc.tensor.matmul(pcf, ehat_sb, a_prev, start=True, stop=True)
            a_new = apool.tile([K, BS], bf16, name="af", tag="af")
            nc.vector.tensor_mul(a_new, pcf, xsl(xF, r))
            a_prev = a_new
            pcv = psum_c.tile([K, BS], f32, name="pcv", tag="pcv")
            nc.tensor.matmul(pcv, ebwd_sb, v_prev, start=True, stop=True)
            v_new = apool.tile([K, BS], bf16, name="av", tag="av")
            nc.vector.tensor_mul(v_new, pcv, xsl(xB, r))
            v_prev = v_new

        nc.sync.dma_start(out=amid_d[:, :], in_=a_prev)
        nc.gpsimd.dma_start(out=vmid_d[:, :], in_=v_prev)

    nc.compile()
    return nc


def _numpy_fallback(emissions, W, b, start_transitions, transitions,
                    end_transitions, tags, mask):
    # Exact replication of the reference semantics (used only if mask is not
    # all-ones, which the spec's input fill guarantees never happens).
    e = emissions.astype(np.float64)
    logits = e @ W.astype(np.float64) + b.astype(np.float64)
    mf = mask.astype(np.float64)
    st = start_transitions.astype(np.float64)
    tr = transitions.astype(np.float64)
    en = end_transitions.astype(np.float64)
    Bn = logits.shape[0]
    bar = np.arange(Bn)
    first = tags[:, 0]
    score = st[first] + logits[bar, 0, first]
    prev = first.copy()
    for t in range(1, T):
        tg = tags[:, t]
        stepv = tr[prev, tg] + logits[bar, t, tg]
        score = score + stepv * mf[:, t]
        prev = np.where(mf[:, t] > 0, tg, prev)
    score = score + en[prev]
    alpha = st[None, :] + logits[:, 0]
    for t in range(1, T):
        nxt = alpha[:, :, None] + tr[None, :, :]
        m = nxt.max(axis=1, keepdims=True)
        nxt = np.log(np.exp(nxt - m).sum(axis=1)) + m[:, 0, :] + logits[:, t]
        alpha = np.where(mf[:, t:t + 1] > 0, nxt, alpha)
    fin = alpha + en[None, :]
    m = fin.max(axis=1, keepdims=True)
    logz = np.log(np.exp(fin - m).sum(axis=1)) + m[:, 0]
    return np.asarray((score - logz).sum(), dtype=np.float32)


def kernel(emissions, W, b, start_transitions, transitions, end_transitions,
           tags, mask):
    global LAST_RESULTS
    emissions = np.ascontiguousarray(np.asarray(emissions, dtype=np.float32))
    W = np.asarray(W, dtype=np.float32)
    b = np.asarray(b, dtype=np.float32)
    start_transitions = np.asarray(start_transitions, dtype=np.float32)
    transitions = np.asarray(transitions, dtype=np.float32)
    end_transitions = np.asarray(end_transitions, dtype=np.float32)
    tags = np.asarray(tags).astype(np.int64)
    mask = np.asarray(mask).astype(bool)

    if not mask.all():
        return _numpy_fallback(emissions, W, b, start_transitions, transitions,
                               end_transitions, tags, mask)

    import ml_dtypes
    from concourse.bass_utils import run_bass_kernel_spmd

    if "nc" not in _BUILT:
        _BUILT["nc"] = _build_nc()
    nc = _BUILT["nc"]

    bf = ml_dtypes.bfloat16
    E32 = np.exp(transitions).astype(np.float32)
    cbig_h = np.zeros((128, 4 * K), dtype=bf)
    cbig_h[:, 0:K] = W[:128].astype(bf)
    cbig_h[:, K:2 * K] = W[128:].astype(bf)
    cbig_h[:K, 2 * K:3 * K] = E32.astype(bf)
    cbig_h[:K, 3 * K:4 * K] = E32.T.astype(bf)
    cflt_h = np.zeros((K, 4), dtype=np.float32)
    cflt_h[:, 0] = b - np.float32(PRE)
    cflt_h[:, 1] = np.exp(start_transitions)
    cflt_h[:, 2] = np.exp(end_transitions)

    in_maps = []
    for c in range(NCORES):
        sh = emissions[c * BS:(c + 1) * BS]              # [BS, T, H]
        # [H, Thalf, BS] -> [128, 2, 8192] (h-half as middle dim)
        ef = np.ascontiguousarray(
            sh[:, :T // 2].transpose(2, 1, 0).reshape(2, 128, HALF)
            .transpose(1, 0, 2)).astype(bf)
        eb = np.ascontiguousarray(
            sh[:, :T // 2 - 1:-1].transpose(2, 1, 0).reshape(2, 128, HALF)
            .transpose(1, 0, 2)).astype(bf)
        in_maps.append(dict(emisF=ef, emisB=eb, cbig=cbig_h, cflt=cflt_h))

    res = run_bass_kernel_spmd(nc, in_maps, list(range(NCORES)))
    LAST_RESULTS = res

    # ---- host finish (fp64) ----
    E64 = np.exp(transitions.astype(np.float64))
    st64 = start_transitions.astype(np.float64)
    tr64 = transitions.astype(np.float64)
    en64 = end_transitions.astype(np.float64)
    b64 = b.astype(np.float64)
    Wt = W.T.astype(np.float64)                          # [K, H]
    logz_corr = T * float(np.float32(PRE))

    total = 0.0
    for c in range(NCORES):
        out = res.results[c]
        amid = out["amid"].astype(np.float64)            # [K, BS]
        vmid = out["vmid"].astype(np.float64)
        zmid = np.einsum("kb,kj,jb->b", amid, E64, vmid)
        logz = np.log(zmid) + logz_corr
        sh = emissions[c * BS:(c + 1) * BS].astype(np.float64)
        tg = tags[c * BS:(c + 1) * BS]
        gold = np.einsum("bth,bth->", sh, Wt[tg])        # emission part
        hterm = (st64[tg[:, 0]].sum()
                 + tr64[tg[:, :-1], tg[:, 1:]].sum()
                 + en64[tg[:, -1]].sum()
                 + b64[tg].sum())
        total += gold + hterm - logz.sum()

    return np.asarray(total, dtype=np.float32)


# revision 21
# speedup vs baseline: 1.0753x; 1.0753x over previous
"""CRF log-likelihood loss kernel for Trainium2 (8 NeuronCores, Bass/Tile).

Strategy (data-parallel over batch, per sharding hint):
  - B=256 batch rows sharded 32 per core; W/b/CRF tables replicated.
  - Bulk: X = exp(W^T e + b - PRE) computed as 16 big bf16 matmuls
    (1024-token chunks, PSUM-accumulated over the two 128-row h-halves)
    + ACT Exp with bias fused.  Runs dense up front so the PE p-state
    ramps to full speed.  Emissions are pre-transposed/cast to bf16 on
    host; fwd tokens ascending and bwd tokens descending are separate
    streams so chain consumption is position-aligned.
  - Chain: fwd alpha from t=0 and bwd beta from t=511 as two interleaved
    32-partition recurrences (one [32,32] matmul + one elementwise
    multiply each per round, phase-locked so PE/DVE overlap).  A constant
    prescale exp(-PRE) per step is folded into the bias so NO
    renormalization is needed (fp32/bf16 exponent range absorbs the
    drift); host adds T*PRE back to logZ exactly.
  - Host finishes: numerator (gold-path score incl. emission term) in
    numpy fp64; logZ_b = log(alpha_255^T E (x*beta)_256) + T*PRE;
    result = sum_b(score_b - logZ_b), summed over cores.
"""

import numpy as np

B, T, H, K = 256, 512, 256, 32
NCORES = 8
BS = B // NCORES          # 32 batch rows per core
NT = T * BS               # 16384 tokens per core
HALF = NT // 2            # 8192 tokens per direction
CTOK = 1024               # tokens per bulk chunk
NCH = HALF // CTOK        # 8 chunks per direction
TS_PER_CH = CTOK // BS    # 32 t-steps per chunk
NROUND = 255              # fwd t=1..255, bwd t=510..256
PRE = 5.75 * 0.6931471805599453  # per-step prescale (log-domain)

_BUILT = {}
LAST_RESULTS = None


def _build_nc():
    import concourse.bacc as bacc
    import concourse.tile as tile
    from concourse import mybir
    from contextlib import ExitStack

    f32 = mybir.dt.float32
    bf16 = mybir.dt.bfloat16
    Exp = mybir.ActivationFunctionType.Exp
    mult = mybir.AluOpType.mult

    nc = bacc.Bacc("TRN2", target_bir_lowering=False, debug=False,
                   num_devices=NCORES)

    # [128, 2 (h-half), 8192] bf16 per direction; fwd t asc, bwd t desc.
    emisF = nc.declare_dram_parameter("emisF", [128, 2, HALF], bf16,
                                      isOutput=False)
    emisB = nc.declare_dram_parameter("emisB", [128, 2, HALF], bf16,
                                      isOutput=False)
    # cbig: cols [w0 | w1 | ehat | ebwd]; cflt: cols [b-PRE, e^start, e^end]
    cbig = nc.declare_dram_parameter("cbig", [128, 4 * K], bf16,
                                     isOutput=False)
    cflt = nc.declare_dram_parameter("cflt", [K, 4], f32, isOutput=False)
    amid_d = nc.declare_dram_parameter("amid", [K, BS], bf16, isOutput=True)
    vmid_d = nc.declare_dram_parameter("vmid", [K, BS], bf16, isOutput=True)

    with ExitStack() as ctx:
        tc = ctx.enter_context(tile.TileContext(nc))
        consts = ctx.enter_context(tc.tile_pool(name="consts", bufs=1))
        emis_pool = ctx.enter_context(tc.tile_pool(name="emis", bufs=8))
        xpool = ctx.enter_context(tc.tile_pool(name="xp", bufs=2 * NCH))
        apool = ctx.enter_context(tc.tile_pool(name="ap", bufs=8))
        psum_l = ctx.enter_context(tc.tile_pool(name="pl", bufs=4,
                                                space="PSUM"))
        psum_c = ctx.enter_context(tc.tile_pool(name="pc", bufs=2,
                                                space="PSUM"))

        # constants: packed into 2 DMAs on the (otherwise idle) scalar
        # queue so the emission chunk DMAs lead the sync/gpsimd queues.
        cb_sb = consts.tile([128, 4 * K], bf16)
        cf_sb = consts.tile([K, 4], f32)
        nc.scalar.dma_start(out=cb_sb, in_=cbig[:, :])
        nc.scalar.dma_start(out=cf_sb, in_=cflt[:, :])
        w0 = cb_sb[:, 0:K]
        w1 = cb_sb[:, K:2 * K]
        ehat_sb = cb_sb[0:K, 2 * K:3 * K]
        ebwd_sb = cb_sb[0:K, 3 * K:4 * K]
        b_sb = cf_sb[:, 0:1]
        estart_sb = cf_sb[:, 1:2]
        eend_sb = cf_sb[:, 2:3]

        # ---- bulk: X = exp(logits + b - PRE), 1024-token chunks.
        # Chunk 0 is produced up front; chunks 1..7 interleave into the
        # chain (one PE matmul per job slot) so DMA/PE bulk overlaps the
        # latency-bound recurrence. ----
        xF = [None] * NCH
        xB = [None] * NCH

        def emit_dma(c, spread=False):
            # 4 subtile DMAs (F s0, F s1, B s0, B s1), each [128, 2x512]
            # (h0|h1).  spread=True fans them over 4 queues (chunk 0).
            qs = ((nc.sync, nc.scalar, nc.gpsimd, nc.scalar) if spread
                  else (nc.sync, nc.sync, nc.gpsimd, nc.gpsimd))
            sub = []
            for i, (src, nm) in enumerate(((emisF, "ef"), (emisB, "eb"))):
                xt = xpool.tile([K, CTOK], bf16, name="xt", tag="xt")
                for s in range(2):
                    off = c * CTOK + s * 512
                    e = emis_pool.tile([128, 1024], bf16, name=nm, tag=nm)
                    qs[2 * i + s].dma_start(
                        out=e, in_=src[:, :, off:off + 512])
                    sub.append(e)
                (xF if i == 0 else xB)[c] = (sub[2 * i:], xt)

        def mm_jobs(c):
            # 8 single-matmul jobs for chunk c (F/B x 2 subtiles x 2 h)
            for xs in (xF, xB):
                sub, xt = xs[c]
                for s in range(2):
                    e = sub[s]
                    ss = slice(s * 512, (s + 1) * 512)
                    pl = psum_l.tile([K, 512], f32, name="pl", tag="pl")

                    def j1(pl=pl, e=e):
                        nc.tensor.matmul(pl, w0, e[:, :512],
                                         start=True, stop=False)

                    def j2(pl=pl, e=e, xt=xt, ss=ss):
                        nc.tensor.matmul(pl, w1, e[:, 512:],
                                         start=False, stop=True)
                        nc.scalar.activation(out=xt[:, ss], in_=pl,
                                             func=Exp, bias=b_sb)

                    yield j1
                    yield j2

        emit_dma(0, spread=True)
        emit_dma(1, spread=True)
        for c in range(NCH):
            if c + 2 < NCH:
                emit_dma(c + 2)
            for job in mm_jobs(c):
                job()

        def xsl(xs, r):
            c, o = r // TS_PER_CH, r % TS_PER_CH
            return xs[c][1][:, o * BS:(o + 1) * BS]

        # ---- chain init: a0 = X_f(0)*estart ; v0 = X_b(0)*eend ----
        a_prev = apool.tile([K, BS], bf16, name="af", tag="af")
        nc.vector.tensor_scalar(out=a_prev, in0=xsl(xF, 0),
                                scalar1=estart_sb, scalar2=None, op0=mult)
        v_prev = apool.tile([K, BS], bf16, name="av", tag="av")
        nc.vector.tensor_scalar(out=v_prev, in0=xsl(xB, 0),
                                scalar1=eend_sb, scalar2=None, op0=mult)

        # ---- 255 interleaved rounds ----
        for r in range(1, NROUND + 1):
            pcf = psum_c.tile([K, BS], f32, name="pcf", tag="pcf")
            nc.tensor.matmul(pcf, ehat_sb, a_prev, start=True, stop=True)
            a_new = apool.tile([K, BS], bf16, name="af", tag="af")
            nc.vector.tensor_mul(a_new, pcf, xsl(xF, r))
            a_prev = a_new
            pcv = psum_c.tile([K, BS], f32, name="pcv", tag="pcv")
            nc.tensor.matmul(pcv, ebwd_sb, v_prev, start=True, stop=True)
            v_new = apool.tile([K, BS], bf16, name="av", tag="av")
            nc.vector.tensor_mul(v_new, pcv, xsl(xB, r))
            v_prev = v_new

        nc.sync.dma_start(out=amid_d[:, :], in_=a_prev)
        nc.gpsimd.dma_start(out=vmid_d[:, :], in_=v_prev)

    nc.compile()
    return nc


def _numpy_fallback(emissions, W, b, start_transitions, transitions,
                    end_transitions, tags, mask):
    # Exact replication of the reference semantics (used only if mask is not
    # all-ones, which the spec's input fill guarantees never happens).
    e = emissions.astype(np.float64)
    logits = e @ W.astype(np.float64) + b.astype(np.float64)
    mf = mask.astype(np.float64)
    st = start_transitions.astype(np.float64)
    tr = transitions.astype(np.float64)
    en = end_transitions.astype(np.float64)
    Bn = logits.shape[0]
    bar = np.arange(Bn)
    first = tags[:, 0]
    score = st[first] + logits[bar, 0, first]
    prev = first.copy()
    for t in range(1, T):
        tg = tags[:, t]
        stepv = tr[prev, tg] + logits[bar, t, tg]
        score = score + stepv * mf[:, t]
        prev = np.where(mf[:, t] > 0, tg, prev)
    score = score + en[prev]
    alpha = st[None, :] + logits[:, 0]
    for t in range(1, T):
        nxt = alpha[:, :, None] + tr[None, :, :]
        m = nxt.max(axis=1, keepdims=True)
        nxt = np.log(np.exp(nxt - m).sum(axis=1)) + m[:, 0, :] + logits[:, t]
        alpha = np.where(mf[:, t:t + 1] > 0, nxt, alpha)
    fin = alpha + en[None, :]
    m = fin.max(axis=1, keepdims=True)
    logz = np.log(np.exp(fin - m).sum(axis=1)) + m[:, 0]
    return np.asarray((score - logz).sum(), dtype=np.float32)


def kernel(emissions, W, b, start_transitions, transitions, end_transitions,
           tags, mask):
    global LAST_RESULTS
    emissions = np.ascontiguousarray(np.asarray(emissions, dtype=np.float32))
    W = np.asarray(W, dtype=np.float32)
    b = np.asarray(b, dtype=np.float32)
    start_transitions = np.asarray(start_transitions, dtype=np.float32)
    transitions = np.asarray(transitions, dtype=np.float32)
    end_transitions = np.asarray(end_transitions, dtype=np.float32)
    tags = np.asarray(tags).astype(np.int64)
    mask = np.asarray(mask).astype(bool)

    if not mask.all():
        return _numpy_fallback(emissions, W, b, start_transitions, transitions,
                               end_transitions, tags, mask)

    import ml_dtypes
    from concourse.bass_utils import run_bass_kernel_spmd

    if "nc" not in _BUILT:
        _BUILT["nc"] = _build_nc()
    nc = _BUILT["nc"]

    bf = ml_dtypes.bfloat16
    E32 = np.exp(transitions).astype(np.float32)
    cbig_h = np.zeros((128, 4 * K), dtype=bf)
    cbig_h[:, 0:K] = W[:128].astype(bf)
    cbig_h[:, K:2 * K] = W[128:].astype(bf)
    cbig_h[:K, 2 * K:3 * K] = E32.astype(bf)
    cbig_h[:K, 3 * K:4 * K] = E32.T.astype(bf)
    cflt_h = np.zeros((K, 4), dtype=np.float32)
    cflt_h[:, 0] = b - np.float32(PRE)
    cflt_h[:, 1] = np.exp(start_transitions)
    cflt_h[:, 2] = np.exp(end_transitions)

    in_maps = []
    for c in range(NCORES):
        sh = emissions[c * BS:(c + 1) * BS]              # [BS, T, H]
        # [H, Thalf, BS] -> [128, 2, 8192] (h-half as middle dim)
        ef = np.ascontiguousarray(
            sh[:, :T // 2].transpose(2, 1, 0).reshape(2, 128, HALF)
            .transpose(1, 0, 2)).astype(bf)
        eb = np.ascontiguousarray(
            sh[:, :T // 2 - 1:-1].transpose(2, 1, 0).reshape(2, 128, HALF)
            .transpose(1, 0, 2)).astype(bf)
        in_maps.append(dict(emisF=ef, emisB=eb, cbig=cbig_h, cflt=cflt_h))

    res = run_bass_kernel_spmd(nc, in_maps, list(range(NCORES)))
    LAST_RESULTS = res

    # ---- host finish (fp64) ----
    E64 = np.exp(transitions.astype(np.float64))
    st64 = start_transitions.astype(np.float64)
    tr64 = transitions.astype(np.float64)
    en64 = end_transitions.astype(np.float64)
    b64 = b.astype(np.float64)
    Wt = W.T.astype(np.float64)                          # [K, H]
    logz_corr = T * float(np.float32(PRE))

    total = 0.0
    for c in range(NCORES):
        out = res.results[c]
        amid = out["amid"].astype(np.float64)            # [K, BS]
        vmid = out["vmid"].astype(np.float64)
        zmid = np.einsum("kb,kj,jb->b", amid, E64, vmid)
        logz = np.log(zmid) + logz_corr
        sh = emissions[c * BS:(c + 1) * BS].astype(np.float64)
        tg = tags[c * BS:(c + 1) * BS]
        gold = np.einsum("bth,bth->", sh, Wt[tg])        # emission part
        hterm = (st64[tg[:, 0]].sum()
                 + tr64[tg[:, :-1], tg[:, 1:]].sum()
                 + en64[tg[:, -1]].sum()
                 + b64[tg].sum())
        total += gold + hterm - logz.sum()

    return np.asarray(total, dtype=np.float32)
